# revision 6
# baseline (speedup 1.0000x reference)
"""Contextual-attention kernel for Trainium2, 8 NeuronCores, SPMD.

Decomposition (validated against the jax reference in numpy):
  scores[l,p] = rn[l] * sum_kk V[kk,l] * Gbox[kk,p]      (matmul1, kk=9*128)
  E = exp(scores - max_l scores)                          (softmax numerator)
  Mz[p,:] = sum_l E[l,p] * [rn[l]*V_lkk[l,:1152], 1]      (matmul2, Z in last col)
  out = col2im(Mz[:, :1152]/Z) * m/9 + fg*(1-m)           (host)

Sharding: core c handles sample c//2, pixel half c%2 (2048 of 4096 pixels).
No collectives; host scatters inputs / gathers outputs.
"""
import sys
for _p in ('/opt/trn_rl_repo',):
    if _p not in sys.path:
        sys.path.insert(0, _p)

import numpy as np

import concourse.bass as bass
import concourse.mybir as mybir
import concourse.tile as tile
from concourse import bacc
from concourse.bass_isa import ReduceOp
from concourse.bass_utils import run_bass_kernel_spmd

EPS = 1e-7
C, H, W = 128, 64, 64
L = H * W                      # 4096
KK = 9 * C                     # 1152
NC_COUNT = 8
HALF = L // 2                  # 2048 pixels per core
NCHUNK = 4                     # p-chunks of 512 per core
CW = 512                       # chunk width (pixels)
LT = 32                        # l-tiles of 128
PT_PER_CORE = 16               # p-tiles of 128 per core
DT_MM = mybir.dt.float32  # exact; float32r needs producer-side rounding
F32 = mybir.dt.float32

_compiled = None


def _build_program():
    nc = bacc.Bacc("TRN2", target_bir_lowering=False, debug=False)
    vslab_d = nc.dram_tensor("vslab", [C, 3 * 66 * 64], F32, kind="ExternalInput").ap()
    rnt_d = nc.dram_tensor("rnt", [C, LT], F32, kind="ExternalInput").ap()
    gsh_d = nc.dram_tensor("gsh", [9, C, HALF], F32, kind="ExternalInput").ap()
    vlkk2_d = nc.dram_tensor("vlkk2", [LT, C, KK + 1], F32, kind="ExternalInput").ap()
    mout_d = nc.dram_tensor("mout", [PT_PER_CORE, C, KK + 1], F32,
                            kind="ExternalOutput").ap()
    ident_d = nc.dram_tensor("ident", [C, C], F32, kind="ExternalInput").ap()
    ones1_d = nc.dram_tensor("ones1", [1, C], F32, kind="ExternalInput").ap()

    with tile.TileContext(nc) as tc:
        with (
            tc.tile_pool(name="const", bufs=1) as cpool,
            tc.tile_pool(name="gpool", bufs=2) as gpool,
            tc.tile_pool(name="sspool", bufs=1) as sspool,
            tc.tile_pool(name="small", bufs=2) as small,
            tc.tile_pool(name="vbufs", bufs=4) as vpool,
            tc.tile_pool(name="mo", bufs=4) as mopool,
            tc.tile_pool(name="ps1", bufs=2, space="PSUM") as ps1,
            tc.tile_pool(name="psm", bufs=2, space="PSUM") as psm,
            tc.tile_pool(name="ps2", bufs=4, space="PSUM") as ps2,
        ):
            vs = cpool.tile([C, 3 * 66 * 64], F32)
            nc.sync.dma_start(out=vs[:], in_=vslab_d[:])
            rnt = cpool.tile([C, LT], F32)
            nc.sync.dma_start(out=rnt[:], in_=rnt_d[:])
            ident = cpool.tile([C, C], F32)
            nc.sync.dma_start(out=ident[:], in_=ident_d[:])
            ones1 = cpool.tile([1, C], F32)
            nc.sync.dma_start(out=ones1[:], in_=ones1_d[:])

            for ch in range(NCHUNK):
                # ---- load G chunk: [128, 9, 512]
                gt = gpool.tile([C, 9, CW], F32, tag="gt")
                for k in range(9):
                    nc.sync.dma_start(out=gt[:, k, :],
                                      in_=gsh_d[k, :, ch * CW:(ch + 1) * CW])

                # ---- matmul1: ss[l, p] for all 32 l-tiles of this chunk
                ss = sspool.tile([C, LT * CW], F32, tag="ss")
                for lt in range(LT):
                    ps = ps1.tile([C, CW], F32, tag="ps1")
                    for k in range(9):
                        di, dj = k // 3, k % 3
                        base = (dj * 66 + 2 * lt + di) * 64
                        lhsT = vs[:, base:base + 128]
                        nc.tensor.matmul(ps[:], lhsT.bitcast(DT_MM),
                                         gt[:, k, :].bitcast(DT_MM),
                                         start=(k == 0), stop=(k == 8))
                    # drain with per-partition rn scale
                    nc.vector.tensor_scalar(
                        out=ss[:, lt * CW:(lt + 1) * CW], in0=ps[:],
                        scalar1=rnt[:, lt:lt + 1], scalar2=None,
                        op0=mybir.AluOpType.mult)

                # ---- max over l (32 tiles then across partitions)
                mrun = small.tile([C, CW], F32, tag="mrun")
                nc.vector.tensor_copy(out=mrun[:], in_=ss[:, 0:CW])
                for lt in range(1, LT):
                    nc.vector.tensor_tensor(out=mrun[:], in0=mrun[:],
                                            in1=ss[:, lt * CW:(lt + 1) * CW],
                                            op=mybir.AluOpType.max)
                # cross-partition max via PE: per 128-px block, transpose,
                # free-axis max, transpose back, ones-broadcast to all partitions
                mb = small.tile([C, CW], F32, tag="mb", name=f"mb_{ch}")
                for b in range(4):
                    tps = psm.tile([C, C], F32, tag="tp", name=f"tp_{ch}_{b}")
                    nc.tensor.transpose(tps[:], mrun[:, b * C:(b + 1) * C], ident[:])
                    tms = small.tile([C, C], F32, tag="tms", name=f"tms_{ch}_{b}")
                    nc.vector.tensor_copy(out=tms[:], in_=tps[:])
                    mcol = small.tile([C, 1], F32, tag="mcol", name=f"mc_{ch}_{b}")
                    nc.vector.tensor_reduce(mcol[:], tms[:],
                                            axis=mybir.AxisListType.XYZW,
                                            op=mybir.AluOpType.max)
                    tp2 = psm.tile([1, C], F32, tag="tp", name=f"tp2_{ch}_{b}")
                    nc.tensor.transpose(tp2[:], mcol[:], ident[:])
                    mrow = small.tile([1, C], F32, tag="mrow", name=f"mr_{ch}_{b}")
                    nc.vector.tensor_copy(out=mrow[:], in_=tp2[:])
                    bps = psm.tile([C, C], F32, tag="tp", name=f"bp_{ch}_{b}")
                    nc.tensor.matmul(bps[:], ones1[:], mrow[:], start=True, stop=True)
                    nc.vector.tensor_copy(out=mb[:, b * C:(b + 1) * C], in_=bps[:])
                mrun = mb

                # ---- exp(ss - m)
                for lt in range(LT):
                    sl = ss[:, lt * CW:(lt + 1) * CW]
                    nc.vector.tensor_tensor(out=sl, in0=sl, in1=mrun[:],
                                            op=mybir.AluOpType.subtract)
                    nc.scalar.activation(sl, sl, mybir.ActivationFunctionType.Exp)

                # ---- matmul2: Mz[p, kk] = sum_l E[l,p] * vlkk2[l,kk]
                for (c0, c1) in ((0, 512), (512, 1024), (1024, KK + 1)):
                    cw = c1 - c0
                    pss = [ps2.tile([C, 512], F32, tag="ps2", name=f"ps2_{ch}_{c0}_{i}")
                           for i in range(4)]
                    for ls in range(LT):
                        vb = vpool.tile([C, 512], F32, tag="vb")
                        nc.sync.dma_start(out=vb[:, :cw], in_=vlkk2_d[ls, :, c0:c1])
                        for pt in range(4):
                            lhsT = ss[:, ls * CW + pt * 128: ls * CW + (pt + 1) * 128]
                            nc.tensor.matmul(pss[pt][:, :cw], lhsT.bitcast(DT_MM),
                                             vb[:, :cw].bitcast(DT_MM),
                                             start=(ls == 0), stop=(ls == LT - 1))
                    for pt in range(4):
                        mo = mopool.tile([C, 512], F32, tag="mo")
                        nc.vector.tensor_copy(out=mo[:, :cw], in_=pss[pt][:, :cw])
                        nc.sync.dma_start(out=mout_d[ch * 4 + pt, :, c0:c1],
                                          in_=mo[:, :cw])
    nc.compile()
    return nc


def _host_prep(fg, m):
    """Per-sample operand tensors. fg [C,H,W] f32, m [1,H,W] f32."""
    bg = fg * (1.0 - m)
    vslab = (np.pad(bg, ((0, 0), (1, 1), (1, 1))) + EPS).astype(np.float32)

    v_lkk = np.empty((L, KK + 1), np.float32)
    for di in range(3):
        for dj in range(3):
            v_lkk[:, (di * 3 + dj) * C:(di * 3 + dj + 1) * C] = \
                vslab[:, di:di + H, dj:dj + W].reshape(C, L).T
    v_lkk[:, KK] = 1.0

    norm2 = np.sum(v_lkk[:, :KK].astype(np.float64) ** 2, axis=1)
    rn = (1.0 / np.sqrt(norm2)).astype(np.float32)
    rnt = np.ascontiguousarray(rn.reshape(LT, C).T)          # [128, 32]

    v_lkk2 = v_lkk.copy()
    v_lkk2[:, :KK] *= rn[:, None]
    vlkk2 = np.ascontiguousarray(v_lkk2.reshape(LT, C, KK + 1))

    fgpad = np.pad(fg, ((0, 0), (1, 1), (1, 1)))
    G = np.empty((9, C, L), np.float32)
    for di in range(3):
        for dj in range(3):
            Z = np.zeros((C, H + 2, W + 2), np.float32)
            Z[:, 1:H + 1, 1:W + 1] = fgpad[:, di:di + H, dj:dj + W]
            B = sum(Z[:, a:a + H, b:b + W] for a in range(3) for b in range(3))
            G[di * 3 + dj] = B.reshape(C, L)
    return vslab, rnt, vlkk2, G


def _host_post(Mpatch, fg, m):
    """col2im + final combine for one sample. Mpatch [L, 1152]."""
    rec = np.zeros((C, H, W), np.float32)
    Mp = Mpatch.reshape(H, W, 9, C)
    for di in range(3):
        for dj in range(3):
            oy, ox = 1 - di, 1 - dj
            ys, ye = max(0, -oy), min(H, H - oy)
            xs, xe = max(0, -ox), min(W, W - ox)
            rec[:, ys:ye, xs:xe] += np.transpose(
                Mp[ys + oy:ye + oy, xs + ox:xe + ox, di * 3 + dj, :], (2, 0, 1))
    return rec * m / 9.0 + fg * (1.0 - m)


def kernel(foreground, mask, _results_hook=None):
    global _compiled
    foreground = np.asarray(foreground, np.float32)
    mask = np.asarray(mask, np.float32)
    B = foreground.shape[0]

    if _compiled is None:
        _compiled = _build_program()
    nc = _compiled

    in_maps = []
    preps = []
    for s in range(B):
        vslab, rnt, vlkk2, G = _host_prep(foreground[s], mask[s])
        # [C,66,66] -> [C, 3(dj), 66, 64]: vs2[c,dj,y,x] = vslab[c,y,x+dj]
        vslab = np.ascontiguousarray(
            np.stack([vslab[:, :, dj:dj + 64] for dj in range(3)], axis=1)
        ).reshape(C, 3 * 66 * 64)
        preps.append((vslab, rnt, vlkk2, G))
    for core in range(NC_COUNT):
        s, h = core // 2, core % 2
        vslab, rnt, vlkk2, G = preps[s]
        in_maps.append({
            "vslab": vslab,
            "rnt": rnt,
            "gsh": np.ascontiguousarray(G[:, :, h * HALF:(h + 1) * HALF]),
            "vlkk2": vlkk2,
            "ident": np.eye(C, dtype=np.float32),
            "ones1": np.ones((1, C), np.float32),
        })

    res = run_bass_kernel_spmd(nc, in_maps, list(range(NC_COUNT)))
    if _results_hook is not None:
        _results_hook(res)

    out = np.empty_like(foreground)
    for s in range(B):
        halves = []
        for h in range(2):
            mo = np.asarray(res.results[2 * s + h]["mout"])      # [16,128,1153]
            halves.append(mo.transpose(0, 1, 2).reshape(HALF, KK + 1))
        Mz = np.concatenate(halves, axis=0)                       # [L, 1153]
        Mpatch = Mz[:, :KK] / Mz[:, KK:KK + 1]
        out[s] = _host_post(Mpatch, foreground[s], mask[s])
    return out
